# revision 5
# baseline (speedup 1.0000x reference)
"""AWGN channel kernel for Trainium2: y = x + sqrt(1/SNR) * noise.

Full inputs x, noise: (16384, 4096) float32. Row-sharded across 8
NeuronCores (data parallel, 2048 rows/core); each core streams 16
chunks of [128, 4096] through SBUF and computes the fused
(noise * STD) + x in one DVE scalar_tensor_tensor op per chunk.
"""

import numpy as np

N_CORES = 8
ROWS, COLS = 16384, 4096
SHARD_ROWS = ROWS // N_CORES  # 2048
P = 128
N_CHUNKS = SHARD_ROWS // P  # 16
SNR = 10.0
STD = float(np.sqrt(1.0 / SNR))

import os

CHUNK_COLS = int(os.environ.get("K_CHUNK_COLS", "4096"))
BUFS = int(os.environ.get("K_BUFS", "3"))
LOAD_ENGINES = os.environ.get("K_LOAD_ENGINES", "sync,sync")  # x,noise
STORE_ENGINE = os.environ.get("K_STORE_ENGINE", "scalar")

_cache = {}


def _build():
    if "nc" in _cache:
        return _cache["nc"]

    import concourse.tile as tile
    from concourse import bacc, mybir

    nc = bacc.Bacc(
        "TRN2",
        target_bir_lowering=False,
        debug=False,
        num_devices=N_CORES,
    )
    x_ap = nc.dram_tensor(
        "x", [SHARD_ROWS, COLS], mybir.dt.float32, kind="ExternalInput"
    ).ap()
    n_ap = nc.dram_tensor(
        "noise", [SHARD_ROWS, COLS], mybir.dt.float32, kind="ExternalInput"
    ).ap()
    y_ap = nc.dram_tensor(
        "y", [SHARD_ROWS, COLS], mybir.dt.float32, kind="ExternalOutput"
    ).ap()

    eng_x, eng_n = (getattr(nc, e) for e in LOAD_ENGINES.split(","))
    eng_y = getattr(nc, STORE_ENGINE)

    # flat view: partition p holds rows [16p, 16p+16) = 65536 contiguous elems
    total_cols = SHARD_ROWS * COLS // P  # 65536
    x_v = x_ap.rearrange("(p r) f -> p (r f)", p=P)
    n_v = n_ap.rearrange("(p r) f -> p (r f)", p=P)
    y_v = y_ap.rearrange("(p r) f -> p (r f)", p=P)

    # chunk schedule: small head/tail chunks shrink pipeline fill/drain
    head = [int(s) for s in os.environ.get("K_HEAD", "1024,1024,2048").split(",") if s]
    tail = [int(s) for s in os.environ.get("K_TAIL", "2048,1024,1024").split(",") if s]
    mid_total = total_cols - sum(head) - sum(tail)
    assert mid_total % CHUNK_COLS == 0, (mid_total, CHUNK_COLS)
    sizes = head + [CHUNK_COLS] * (mid_total // CHUNK_COLS) + tail

    with tile.TileContext(nc) as tc:
        with (
            tc.tile_pool(name="xp", bufs=BUFS) as xp,
            tc.tile_pool(name="npool", bufs=BUFS) as npool,
            tc.tile_pool(name="yp", bufs=BUFS) as yp,
        ):
            off = 0
            for w in sizes:
                xt = xp.tile([P, w], mybir.dt.float32, tag="xt")
                nt = npool.tile([P, w], mybir.dt.float32, tag="nt")
                yt = yp.tile([P, w], mybir.dt.float32, tag="yt")
                eng_x.dma_start(out=xt[:], in_=x_v[:, off : off + w])
                eng_n.dma_start(out=nt[:], in_=n_v[:, off : off + w])
                nc.vector.scalar_tensor_tensor(
                    out=yt[:],
                    in0=nt[:],
                    scalar=STD,
                    in1=xt[:],
                    op0=mybir.AluOpType.mult,
                    op1=mybir.AluOpType.add,
                )
                eng_y.dma_start(out=y_v[:, off : off + w], in_=yt[:])
                off += w
            assert off == total_cols

    nc.compile()
    _cache["nc"] = nc
    return nc


def _run(x, noise, trace=False, tmpdir=None):
    from concourse.bass_utils import run_bass_kernel_spmd

    nc = _build()
    x = np.ascontiguousarray(x, dtype=np.float32)
    noise = np.ascontiguousarray(noise, dtype=np.float32)
    in_maps = [
        {
            "x": x[i * SHARD_ROWS : (i + 1) * SHARD_ROWS],
            "noise": noise[i * SHARD_ROWS : (i + 1) * SHARD_ROWS],
        }
        for i in range(N_CORES)
    ]
    res = run_bass_kernel_spmd(
        nc, in_maps, list(range(N_CORES)), trace=trace, tmpdir=tmpdir
    )
    out = np.concatenate([res.results[i]["y"] for i in range(N_CORES)], axis=0)
    return out, res


def kernel(x, noise):
    out, _ = _run(x, noise)
    return out


# revision 6
# speedup vs baseline: 1.1973x; 1.1973x over previous
"""AWGN channel kernel for Trainium2: y = x + sqrt(1/SNR) * noise.

Full inputs x, noise: (16384, 4096) float32. Row-sharded across 8
NeuronCores (data parallel, 2048 rows/core); each core streams 16
chunks of [128, 4096] through SBUF and computes the fused
(noise * STD) + x in one DVE scalar_tensor_tensor op per chunk.
"""

import numpy as np

N_CORES = 8
ROWS, COLS = 16384, 4096
SHARD_ROWS = ROWS // N_CORES  # 2048
P = 128
N_CHUNKS = SHARD_ROWS // P  # 16
SNR = 10.0
STD = float(np.sqrt(1.0 / SNR))

import os

CHUNK_COLS = int(os.environ.get("K_CHUNK_COLS", "4096"))
BUFS = int(os.environ.get("K_BUFS", "3"))
LOAD_ENGINES = os.environ.get("K_LOAD_ENGINES", "sync,sync")  # x,noise
STORE_ENGINE = os.environ.get("K_STORE_ENGINE", "scalar")

_cache = {}


def _build():
    if "nc" in _cache:
        return _cache["nc"]

    import concourse.tile as tile
    from concourse import bacc, mybir

    nc = bacc.Bacc(
        "TRN2",
        target_bir_lowering=False,
        debug=False,
        num_devices=N_CORES,
    )
    x_ap = nc.dram_tensor(
        "x", [SHARD_ROWS, COLS], mybir.dt.float32, kind="ExternalInput"
    ).ap()
    n_ap = nc.dram_tensor(
        "noise", [SHARD_ROWS, COLS], mybir.dt.float32, kind="ExternalInput"
    ).ap()
    y_ap = nc.dram_tensor(
        "y", [SHARD_ROWS, COLS], mybir.dt.float32, kind="ExternalOutput"
    ).ap()

    eng_x, eng_n = (getattr(nc, e) for e in LOAD_ENGINES.split(","))
    eng_y = getattr(nc, STORE_ENGINE)

    # row-block view: block c = rows [128c, 128c+128) — fully contiguous 2 MiB
    x_v = x_ap.rearrange("(c p) f -> c p f", p=P)
    n_v = n_ap.rearrange("(c p) f -> c p f", p=P)
    y_v = y_ap.rearrange("(c p) f -> c p f", p=P)
    n_blocks = SHARD_ROWS // P  # 16

    # chunk schedule: (block, col_off, width). First/last blocks are split
    # into smaller column chunks to shrink pipeline fill/drain.
    def _splits(env, default):
        v = os.environ.get(env, default)
        out = [int(s) for s in v.split(",") if s]
        assert not out or sum(out) == COLS, out
        return out

    head = _splits("K_HEAD", "1024,1024,2048")
    tail = _splits("K_TAIL", "2048,1024,1024")
    chunks = []
    for c in range(n_blocks):
        if c == 0 and head:
            splits = head
        elif c == n_blocks - 1 and tail:
            splits = tail
        else:
            splits = [CHUNK_COLS] * (COLS // CHUNK_COLS)
        off = 0
        for w in splits:
            chunks.append((c, off, w))
            off += w

    with tile.TileContext(nc) as tc:
        with (
            tc.tile_pool(name="xp", bufs=BUFS) as xp,
            tc.tile_pool(name="npool", bufs=BUFS) as npool,
            tc.tile_pool(name="yp", bufs=BUFS) as yp,
        ):
            for c, off, w in chunks:
                xt = xp.tile([P, w], mybir.dt.float32, tag="xt")
                nt = npool.tile([P, w], mybir.dt.float32, tag="nt")
                yt = yp.tile([P, w], mybir.dt.float32, tag="yt")
                eng_x.dma_start(out=xt[:], in_=x_v[c, :, off : off + w])
                eng_n.dma_start(out=nt[:], in_=n_v[c, :, off : off + w])
                nc.vector.scalar_tensor_tensor(
                    out=yt[:],
                    in0=nt[:],
                    scalar=STD,
                    in1=xt[:],
                    op0=mybir.AluOpType.mult,
                    op1=mybir.AluOpType.add,
                )
                eng_y.dma_start(out=y_v[c, :, off : off + w], in_=yt[:])

    nc.compile()
    _cache["nc"] = nc
    return nc


def _run(x, noise, trace=False, tmpdir=None):
    from concourse.bass_utils import run_bass_kernel_spmd

    nc = _build()
    x = np.ascontiguousarray(x, dtype=np.float32)
    noise = np.ascontiguousarray(noise, dtype=np.float32)
    in_maps = [
        {
            "x": x[i * SHARD_ROWS : (i + 1) * SHARD_ROWS],
            "noise": noise[i * SHARD_ROWS : (i + 1) * SHARD_ROWS],
        }
        for i in range(N_CORES)
    ]
    res = run_bass_kernel_spmd(
        nc, in_maps, list(range(N_CORES)), trace=trace, tmpdir=tmpdir
    )
    out = np.concatenate([res.results[i]["y"] for i in range(N_CORES)], axis=0)
    return out, res


def kernel(x, noise):
    out, _ = _run(x, noise)
    return out
